# revision 11
# baseline (speedup 1.0000x reference)
"""Causal (diagonal=1) single-head attention for trn2, 8-core SPMD.

Reference computation (fp32):
    k = key @ Wk.T; q = query @ Wq.T; v = value @ Wv.T       # [B,T,H]
    qk = (q @ k.T) / sqrt(E)                                  # [B,T,T]
    qk masked with tril(ones, k=1) and padding_mask           # -inf outside
    attn = softmax(qk, -1) @ v                                # [B,T,H]

Sharding: data-parallel over batch, 2 batches per core, no collectives.

Algebraic cut: q @ k.T = (query @ Wq.T)(key @ Wk.T).T
            = query @ (Wq.T @ Wk) @ key.T = query @ G @ key.T,
with G = Wq.T @ Wk precomputed once on the host.  The device then runs
ONE projection (A = query @ G) instead of two, and the scores matmul
consumes keyT straight from its DMA layout -- the whole k-projection
(1024^3 MACs/batch) disappears.

Device kernel (per core, per batch), all matmuls fp16 with fp32 PSUM
(fp16's 10 mantissa bits beat bf16's 8; same PE throughput):
    AT[e,t]  = sum_ec G[ec][:,e-chunk].T @ queryT[ec][:,t]    (proj)
    v[s,h]   = valueT[ec][:,s-chunk].T @ Wv.T[ec][:,h]
    sT[s,t]  = keyT-chunk.T @ AT  (only causally-live s-chunks)
    pT[s,t]  = exp(sT/32)  (ScalarE; max-subtraction skipped: |s/32| <~ 6)
    pT       = affine_select(pT, keep j<=i+1, else 0)         (GPSIMD)
    num[t,h] = pT-chunk.T @ v ; den[t,1] = pT-chunk.T @ pad01
    out[t,h] = num * reciprocal(den)                          (VectorE)

padding_mask is folded in exactly on the host: v rows and the denominator
column are scaled by pad01 = (padding_mask == 0), which equals softmax
with -inf at padded keys.
"""
from contextlib import ExitStack

import numpy as np

import concourse.bass as bass
import concourse.mybir as mybir
import concourse.tile as tile
from concourse.bass_utils import run_bass_kernel_spmd

F16 = mybir.dt.float16
F32 = mybir.dt.float32
P = 128
T = 1024           # sequence length
E = 1024           # embed dim
H = 1024           # head dim
NB = 16            # full batch
NCORES = 8
BPC = NB // NCORES  # batches per core
NC = T // P        # 128-chunks per dim (8)
SCALE = 1.0 / 32.0  # 1/sqrt(E)

_nc_cache = None


# --- walrus workaround: one sync-wait per instruction ---------------------
def _split_multi_waits(nc):
    """This walrus build rejects instructions with >1 sync wait (2 for
    EventSemaphore).  Move extra waits onto fresh same-engine NOPs placed
    immediately before the instruction; per-engine in-order execution
    preserves the gating, and semaphore updates stay on the original."""
    for fn in nc.m.functions:
        for bb in fn.blocks:
            il = bb.instructions
            idx = 0
            while idx < len(il):
                inst = il[idx]
                si = inst.sync_info
                waits = list(si.on_wait) if si and si.on_wait else []
                cap = 2 if isinstance(inst, mybir.InstEventSemaphore) else 1
                if len(waits) > cap:
                    extra, keep = waits[:-cap], waits[-cap:]
                    for j, w in enumerate(extra):
                        nop = mybir.InstNoOp(
                            name=f"I-wsplit-{inst.name}-{j}",
                            engine=inst.engine,
                            ins=[],
                            outs=[],
                            sync_info=mybir.SyncInfo(on_wait=[w], on_update=[]),
                        )
                        il.insert(idx, nop)
                        idx += 1
                    inst.sync_info = mybir.SyncInfo(
                        on_wait=keep, on_update=list(si.on_update or [])
                    )
                idx += 1


def _n_sc(ti):
    """Number of live 128-wide s-chunks for t-tile ti (cols j <= t+1)."""
    return min(ti + 2, NC)


def _emit_batch(nc, pools, b, dram):
    Exp = mybir.ActivationFunctionType.Exp
    g_t, w_v = pools["g"], pools["wv"]
    sb, ps, psd = pools["sb"], pools["ps"], pools["psd"]
    first = b == 0

    # -- input loads, spread across engine DMA queues so transfers run in
    #    parallel (each engine's dma_start lands on its own DGE queue and an
    #    engine-issued DMA occupies that engine for the transfer time):
    #      sync:   queryT halves, then keyT, then next batch's loads
    #      vector: G slices ht-major (batch 0), then Wv
    #      gpsimd: valueT st-major slices, padding col
    #    Ordered so the first A-proj group's operands land first. --
    qin = [sb.tile([P, T], F16, name=f"qin{ec}") for ec in range(NC)]
    if first:
        for ht in range(NC):
            for ec in range(NC):
                nc.gpsimd.dma_start(
                    g_t[ec][:, bass.ts(ht, P)],
                    dram["g"][bass.ts(ec, P), bass.ts(ht, P)],
                )
    for ec in range(NC):
        nc.sync.dma_start(qin[ec][:, 0:512], dram["qT"][b, bass.ts(ec, P), 0:512])
    for ec in range(NC):
        nc.sync.dma_start(
            qin[ec][:, 512:1024], dram["qT"][b, bass.ts(ec, P), 512:1024]
        )

    # -- A = query @ G, transposed chunks AT[ht] --
    ATs = [sb.tile([P, T], F16, name=f"ATs{h}") for h in range(NC)]
    for tg in range(2):
        for ht in range(NC):
            acc = ps.tile([P, 512], F32, name="ps")
            for ec in range(NC):
                nc.tensor.matmul(
                    acc[:],
                    lhsT=g_t[ec][:, bass.ts(ht, P)],
                    rhs=qin[ec][:, bass.ts(tg, 512)],
                    start=(ec == 0),
                    stop=(ec == NC - 1),
                )
            nc.scalar.copy(ATs[ht][:, bass.ts(tg, 512)], acc[:])

    # -- v = value @ Wv.T --
    vin = [sb.tile([P, T], F16, name=f"vin{ec}") for ec in range(NC)]
    for st in range(NC):
        for ec in range(NC):
            nc.gpsimd.dma_start(
                vin[ec][:, bass.ts(st, P)],
                dram["vT"][b, bass.ts(ec, P), bass.ts(st, P)],
            )
    if first:
        for ec in range(NC):
            nc.scalar.dma_start(w_v[ec][:], dram["wv"][bass.ts(ec, P), :])
    kin = [sb.tile([P, T], F16, name=f"kin{ec}") for ec in range(NC)]
    for ec in range(NC):
        nc.sync.dma_start(kin[ec][:], dram["kT"][b, bass.ts(ec, P), :])
    padt = sb.tile([P, NC], F16, name="padt", bufs=2)
    nc.gpsimd.dma_start(
        padt[:], dram["pad"][b].rearrange("(c p) x -> p (c x)", p=P)
    )
    v_sb = [sb.tile([P, T], F16, name=f"vsb{s}") for s in range(NC)]
    for st in range(NC):
        for hh in range(2):
            acc = ps.tile([P, 512], F32, name="ps")
            for ec in range(NC):
                nc.tensor.matmul(
                    acc[:],
                    lhsT=vin[ec][:, bass.ts(st, P)],
                    rhs=w_v[ec][:, bass.ts(hh, 512)],
                    start=(ec == 0),
                    stop=(ec == NC - 1),
                )
            nc.vector.tensor_copy(v_sb[st][:, bass.ts(hh, 512)], acc[:])

    # -- scores^T + exp + causal zeroing --
    # Ragged t-groups aligned to where the live s-chunk count jumps
    # (t = 128k - 1 because of the +1 diagonal): 2/4/6/8 live chunks per
    # group = 20 column-blocks vs 23 for aligned 256-wide groups.
    BOUNDS = (0, 255, 511, 767, 1024)
    pT = [sb.tile([P, T], F16, name=f"pT{s}") for s in range(NC)]
    # Blocks that are causally dead but still read by the attn matmuls
    # (created by the ragged grouping) must be zeroed explicitly.
    for sc, pt0, pt1 in ((2, 128, 255), (4, 384, 511), (6, 640, 767)):
        nc.vector.memset(pT[sc][:, pt0:pt1], 0.0)
    for g in range(4):
        t0, t1 = BOUNDS[g], BOUNDS[g + 1]
        w = t1 - t0
        for sc in range(2 * g + 2):
            acc = ps.tile([P, 512], F32, name="ps")
            for ec in range(NC):
                nc.tensor.matmul(
                    acc[:, :w],
                    lhsT=kin[ec][:, bass.ts(sc, P)],
                    rhs=ATs[ec][:, t0:t1],
                    start=(ec == 0),
                    stop=(ec == NC - 1),
                )
            dst = pT[sc][:, t0:t1]
            nc.scalar.activation(dst, acc[:, :w], Exp, scale=SCALE)
            off = 128 * sc - t0
            if off >= -125:
                # keep where t_local - s_local - off + 1 >= 0 (j <= i+1)
                nc.gpsimd.affine_select(
                    out=dst,
                    in_=dst,
                    compare_op=mybir.AluOpType.is_ge,
                    fill=0.0,
                    base=1 - off,
                    pattern=[[1, w]],
                    channel_multiplier=-1,
                )

    # -- attn = (pT.T @ [v, pad01]) with post-normalization --
    for ti in range(NC):
        nsc = _n_sc(ti)
        po0 = ps.tile([P, 512], F32, name="ps")
        po1 = ps.tile([P, 512], F32, name="ps")
        pd = psd.tile([P, 1], F32, name="psd")
        for sc in range(nsc):
            lhsT = pT[sc][:, bass.ts(ti, P)]
            st, sp = (sc == 0), (sc == nsc - 1)
            nc.tensor.matmul(po0[:], lhsT=lhsT, rhs=v_sb[sc][:, 0:512],
                             start=st, stop=sp)
            nc.tensor.matmul(po1[:], lhsT=lhsT, rhs=v_sb[sc][:, 512:1024],
                             start=st, stop=sp)
            nc.tensor.matmul(pd[:], lhsT=lhsT, rhs=padt[:, sc:sc + 1],
                             start=st, stop=sp)
        r = sb.tile([P, 1], F32, name="recip", bufs=3)
        nc.vector.reciprocal(r[:], pd[:])
        osb = sb.tile([P, T], F32, name="osb", bufs=3)
        # the two halves scale concurrently on VectorE and ScalarE; the
        # output DMA is spread over three queues (gpsimd takes the high
        # half, sync/scalar alternate the low half) so no single queue
        # exceeds what it can sustain during the attn phase
        last = b == BPC - 1 and ti == NC - 1
        nstrip = 2 if last else 1  # finer strips shorten the final tail
        sw = 512 // nstrip
        lo_eng = nc.sync if ti % 2 else nc.scalar
        for j in range(nstrip):
            lo, hi = j * sw, (j + 1) * sw
            nc.vector.tensor_scalar_mul(osb[:, lo:hi], po0[:, lo:hi], r[:])
            lo_eng.dma_start(dram["out"][b, bass.ts(ti, P), lo:hi],
                             osb[:, lo:hi])
            nc.scalar.activation(osb[:, 512 + lo:512 + hi], po1[:, lo:hi],
                                 mybir.ActivationFunctionType.Copy, scale=r[:])
            nc.gpsimd.dma_start(dram["out"][b, bass.ts(ti, P), 512 + lo:512 + hi],
                                osb[:, 512 + lo:512 + hi])


def _build_nc():
    nc = bass.Bass()
    dram = {
        "qT": nc.declare_dram_parameter("qT", [BPC, E, T], F16, isOutput=False),
        "kT": nc.declare_dram_parameter("kT", [BPC, E, T], F16, isOutput=False),
        "vT": nc.declare_dram_parameter("vT", [BPC, E, T], F16, isOutput=False),
        "g": nc.declare_dram_parameter("g", [E, E], F16, isOutput=False),
        "wv": nc.declare_dram_parameter("wv", [E, H], F16, isOutput=False),
        "pad": nc.declare_dram_parameter("pad", [BPC, T, 1], F16, isOutput=False),
        "out": nc.declare_dram_parameter("out", [BPC, T, H], F32, isOutput=True),
    }
    with tile.TileContext(nc) as tc, ExitStack() as ctx:
        sb = ctx.enter_context(tc.tile_pool(name="sb", bufs=1))
        ps = ctx.enter_context(tc.tile_pool(name="ps", bufs=6, space="PSUM"))
        psd = ctx.enter_context(tc.tile_pool(name="psd", bufs=2, space="PSUM"))

        pools = {"sb": sb, "ps": ps, "psd": psd}
        pools["g"] = [sb.tile([P, E], F16, name=f"g{ec}") for ec in range(NC)]
        pools["wv"] = [sb.tile([P, H], F16, name=f"wv{ec}") for ec in range(NC)]

        # PE warm-up: dependency-free junk matmuls bridge the initial DMA
        # window (~14 us: DGE priming + first operand transfers) and trip
        # the HAM clock gate to 2.4 GHz before the first real matmul.  Four
        # rotating PSUM tiles, one long accumulation group per tile, so no
        # WAW semaphores serialize the stream.
        NWARM = 32
        warm = sb.tile([P, 512], F16, name="warm")
        nc.vector.memset(warm[:], 0.0)
        wps = [ps.tile([P, 512], F32, name="ps") for _ in range(4)]
        for i in range(NWARM):
            nc.tensor.matmul(wps[i % 4][:], lhsT=warm[:, 0:P], rhs=warm[:],
                             start=(i < 4), stop=(i >= NWARM - 4),
                             skip_group_check=True)

        for b in range(BPC):
            _emit_batch(nc, pools, b, dram)

    _split_multi_waits(nc)
    return nc


def _get_nc():
    global _nc_cache
    if _nc_cache is None:
        _nc_cache = _build_nc()
    return _nc_cache


def _make_in_maps(key, query, value, padding_mask, Wk, Wq, Wv):
    f16 = np.float16
    g = (np.asarray(Wq, np.float32).T @ np.asarray(Wk, np.float32)).astype(f16)
    wv = np.ascontiguousarray(np.asarray(Wv, np.float32).T).astype(f16)
    pad01 = (padding_mask.reshape(NB, T) == 0).astype(np.float32)  # [B,T]
    in_maps = []
    for c in range(NCORES):
        s = slice(BPC * c, BPC * (c + 1))
        qT = np.ascontiguousarray(query[s].transpose(0, 2, 1)).astype(f16)
        kT = np.ascontiguousarray(key[s].transpose(0, 2, 1)).astype(f16)
        vTf = value[s].transpose(0, 2, 1) * pad01[s][:, None, :]
        vT = np.ascontiguousarray(vTf).astype(f16)
        in_maps.append({
            "qT": qT, "kT": kT, "vT": vT,
            "g": g, "wv": wv,
            "pad": pad01[s].astype(f16).reshape(BPC, T, 1),
        })
    return in_maps


def run_on_cores(in_maps, trace=False, **kw):
    nc = _get_nc()
    return run_bass_kernel_spmd(nc, in_maps, list(range(NCORES)), trace=trace, **kw)


def kernel(key, query, value, padding_mask, Wk, Wq, Wv):
    key = np.asarray(key)
    query = np.asarray(query)
    value = np.asarray(value)
    padding_mask = np.asarray(padding_mask)
    in_maps = _make_in_maps(key, query, value, padding_mask,
                            np.asarray(Wk), np.asarray(Wq), np.asarray(Wv))
    res = run_on_cores(in_maps)
    out = np.empty((NB, T, H), np.float32)
    for c in range(NCORES):
        out[BPC * c: BPC * (c + 1)] = res.results[c]["out"]
    return out


# revision 13
# speedup vs baseline: 1.0388x; 1.0388x over previous
"""Causal (diagonal=1) single-head attention for trn2, 8-core SPMD.

Reference computation (fp32):
    k = key @ Wk.T; q = query @ Wq.T; v = value @ Wv.T       # [B,T,H]
    qk = (q @ k.T) / sqrt(E)                                  # [B,T,T]
    qk masked with tril(ones, k=1) and padding_mask           # -inf outside
    attn = softmax(qk, -1) @ v                                # [B,T,H]

Sharding: data-parallel over batch, 2 batches per core, no collectives.

Algebraic cut: q @ k.T = (query @ Wq.T)(key @ Wk.T).T
            = query @ (Wq.T @ Wk) @ key.T = query @ G @ key.T,
with G = Wq.T @ Wk precomputed once on the host.  The device then runs
ONE projection (A = query @ G) instead of two, and the scores matmul
consumes keyT straight from its DMA layout -- the whole k-projection
(1024^3 MACs/batch) disappears.

Device kernel (per core, per batch), all matmuls fp16 with fp32 PSUM
(fp16's 10 mantissa bits beat bf16's 8; same PE throughput):
    AT[e,t]  = sum_ec G[ec][:,e-chunk].T @ queryT[ec][:,t]    (proj)
    v[s,h]   = valueT[ec][:,s-chunk].T @ Wv.T[ec][:,h]
    sT[s,t]  = keyT-chunk.T @ AT  (only causally-live s-chunks)
    pT[s,t]  = exp(sT/32)  (ScalarE; max-subtraction skipped: |s/32| <~ 6)
    pT       = affine_select(pT, keep j<=i+1, else 0)         (GPSIMD)
    num[t,h] = pT-chunk.T @ v ; den[t,1] = pT-chunk.T @ pad01
    out[t,h] = num * reciprocal(den)                          (VectorE)

padding_mask is folded in exactly on the host: v rows and the denominator
column are scaled by pad01 = (padding_mask == 0), which equals softmax
with -inf at padded keys.
"""
from contextlib import ExitStack

import numpy as np

import concourse.bass as bass
import concourse.mybir as mybir
import concourse.tile as tile
from concourse.bass_utils import run_bass_kernel_spmd

F16 = mybir.dt.float16
F32 = mybir.dt.float32
P = 128
T = 1024           # sequence length
E = 1024           # embed dim
H = 1024           # head dim
NB = 16            # full batch
NCORES = 8
BPC = NB // NCORES  # batches per core
NC = T // P        # 128-chunks per dim (8)
SCALE = 1.0 / 32.0  # 1/sqrt(E)

_nc_cache = None


# --- walrus workaround: one sync-wait per instruction ---------------------
def _split_multi_waits(nc):
    """This walrus build rejects instructions with >1 sync wait (2 for
    EventSemaphore).  Move extra waits onto fresh same-engine NOPs placed
    immediately before the instruction; per-engine in-order execution
    preserves the gating, and semaphore updates stay on the original."""
    for fn in nc.m.functions:
        for bb in fn.blocks:
            il = bb.instructions
            idx = 0
            while idx < len(il):
                inst = il[idx]
                si = inst.sync_info
                waits = list(si.on_wait) if si and si.on_wait else []
                cap = 2 if isinstance(inst, mybir.InstEventSemaphore) else 1
                if len(waits) > cap:
                    extra, keep = waits[:-cap], waits[-cap:]
                    for j, w in enumerate(extra):
                        nop = mybir.InstNoOp(
                            name=f"I-wsplit-{inst.name}-{j}",
                            engine=inst.engine,
                            ins=[],
                            outs=[],
                            sync_info=mybir.SyncInfo(on_wait=[w], on_update=[]),
                        )
                        il.insert(idx, nop)
                        idx += 1
                    inst.sync_info = mybir.SyncInfo(
                        on_wait=keep, on_update=list(si.on_update or [])
                    )
                idx += 1


def _n_sc(ti):
    """Number of live 128-wide s-chunks for t-tile ti (cols j <= t+1)."""
    return min(ti + 2, NC)


def _emit_batch(nc, pools, b, dram):
    Exp = mybir.ActivationFunctionType.Exp
    g_t, w_v = pools["g"], pools["wv"]
    sb, ps, psd = pools["sb"], pools["ps"], pools["psd"]
    first = b == 0

    # -- input loads, spread across engine DMA queues so transfers run in
    #    parallel (each engine's dma_start lands on its own DGE queue and an
    #    engine-issued DMA occupies that engine for the transfer time):
    #      sync:   queryT halves, then keyT, then next batch's loads
    #      vector: G slices ht-major (batch 0), then Wv
    #      gpsimd: valueT st-major slices, padding col
    #    Ordered so the first A-proj group's operands land first. --
    qin = [sb.tile([P, T], F16, name=f"qin{ec}") for ec in range(NC)]
    for ec in range(NC):
        nc.sync.dma_start(qin[ec][:, 0:512], dram["qT"][b, bass.ts(ec, P), 0:512])
    if first:
        # whole [128,1024] chunks (2 KB DMA lines; 128-col slices would run
        # at ~1/5 the bandwidth), split 6/2 over gpsimd+sync so both queues
        # finish together just as the warm-up junk drains
        for ec in range(6):
            nc.gpsimd.dma_start(g_t[ec][:], dram["g"][bass.ts(ec, P), :])
        for ec in range(6, NC):
            nc.sync.dma_start(g_t[ec][:], dram["g"][bass.ts(ec, P), :])
    for ec in range(NC):
        nc.sync.dma_start(
            qin[ec][:, 512:1024], dram["qT"][b, bass.ts(ec, P), 512:1024]
        )

    # -- A = query @ G, transposed chunks AT[ht] --
    ATs = [sb.tile([P, T], F16, name=f"ATs{h}") for h in range(NC)]
    for tg in range(2):
        for ht in range(NC):
            acc = ps.tile([P, 512], F32, name="ps")
            for ec in range(NC):
                nc.tensor.matmul(
                    acc[:],
                    lhsT=g_t[ec][:, bass.ts(ht, P)],
                    rhs=qin[ec][:, bass.ts(tg, 512)],
                    start=(ec == 0),
                    stop=(ec == NC - 1),
                )
            nc.scalar.copy(ATs[ht][:, bass.ts(tg, 512)], acc[:])
        if first and tg == 0:
            # Wv lands on scalar's queue here: transfers begin only after
            # the tg=0 copies, keeping the critical head window (G+queryT)
            # uncontended; still arrives well before the v-projection
            for ec in range(NC):
                nc.scalar.dma_start(w_v[ec][:], dram["wv"][bass.ts(ec, P), :])

    # -- v = value @ Wv.T --
    vin = [sb.tile([P, T], F16, name=f"vin{ec}") for ec in range(NC)]
    for ec in range(NC):
        nc.gpsimd.dma_start(vin[ec][:], dram["vT"][b, bass.ts(ec, P), :])
    kin = [sb.tile([P, T], F16, name=f"kin{ec}") for ec in range(NC)]
    for ec in range(NC):
        nc.sync.dma_start(kin[ec][:], dram["kT"][b, bass.ts(ec, P), :])
    padt = sb.tile([P, NC], F16, name="padt", bufs=2)
    nc.gpsimd.dma_start(
        padt[:], dram["pad"][b].rearrange("(c p) x -> p (c x)", p=P)
    )
    v_sb = [sb.tile([P, T], F16, name=f"vsb{s}") for s in range(NC)]
    for st in range(NC):
        for hh in range(2):
            acc = ps.tile([P, 512], F32, name="ps")
            for ec in range(NC):
                nc.tensor.matmul(
                    acc[:],
                    lhsT=vin[ec][:, bass.ts(st, P)],
                    rhs=w_v[ec][:, bass.ts(hh, 512)],
                    start=(ec == 0),
                    stop=(ec == NC - 1),
                )
            nc.vector.tensor_copy(v_sb[st][:, bass.ts(hh, 512)], acc[:])

    # -- scores^T + exp + causal zeroing --
    # Ragged t-groups aligned to where the live s-chunk count jumps
    # (t = 128k - 1 because of the +1 diagonal): 2/4/6/8 live chunks per
    # group = 20 column-blocks vs 23 for aligned 256-wide groups.
    BOUNDS = (0, 255, 511, 767, 1024)
    pT = [sb.tile([P, T], F16, name=f"pT{s}") for s in range(NC)]
    # Blocks that are causally dead but still read by the attn matmuls
    # (created by the ragged grouping) must be zeroed explicitly.
    for sc, pt0, pt1 in ((2, 128, 255), (4, 384, 511), (6, 640, 767)):
        nc.vector.memset(pT[sc][:, pt0:pt1], 0.0)
    for g in range(4):
        t0, t1 = BOUNDS[g], BOUNDS[g + 1]
        w = t1 - t0
        for sc in range(2 * g + 2):
            acc = ps.tile([P, 512], F32, name="ps")
            for ec in range(NC):
                nc.tensor.matmul(
                    acc[:, :w],
                    lhsT=kin[ec][:, bass.ts(sc, P)],
                    rhs=ATs[ec][:, t0:t1],
                    start=(ec == 0),
                    stop=(ec == NC - 1),
                )
            dst = pT[sc][:, t0:t1]
            nc.scalar.activation(dst, acc[:, :w], Exp, scale=SCALE)
            off = 128 * sc - t0
            if off >= -125:
                # keep where t_local - s_local - off + 1 >= 0 (j <= i+1)
                nc.gpsimd.affine_select(
                    out=dst,
                    in_=dst,
                    compare_op=mybir.AluOpType.is_ge,
                    fill=0.0,
                    base=1 - off,
                    pattern=[[1, w]],
                    channel_multiplier=-1,
                )

    # -- attn = (pT.T @ [v, pad01]) with post-normalization --
    for ti in range(NC):
        nsc = _n_sc(ti)
        po0 = ps.tile([P, 512], F32, name="ps")
        po1 = ps.tile([P, 512], F32, name="ps")
        pd = psd.tile([P, 1], F32, name="psd")
        for sc in range(nsc):
            lhsT = pT[sc][:, bass.ts(ti, P)]
            st, sp = (sc == 0), (sc == nsc - 1)
            nc.tensor.matmul(po0[:], lhsT=lhsT, rhs=v_sb[sc][:, 0:512],
                             start=st, stop=sp)
            nc.tensor.matmul(po1[:], lhsT=lhsT, rhs=v_sb[sc][:, 512:1024],
                             start=st, stop=sp)
            nc.tensor.matmul(pd[:], lhsT=lhsT, rhs=padt[:, sc:sc + 1],
                             start=st, stop=sp)
        r = sb.tile([P, 1], F32, name="recip", bufs=3)
        nc.vector.reciprocal(r[:], pd[:])
        osb = sb.tile([P, T], F32, name="osb", bufs=3)
        # the two halves scale concurrently on VectorE and ScalarE; the
        # output DMA is spread over three queues (gpsimd takes the high
        # half, sync/scalar alternate the low half) so no single queue
        # exceeds what it can sustain during the attn phase
        last = b == BPC - 1 and ti == NC - 1
        nstrip = 2 if last else 1  # finer strips shorten the final tail
        sw = 512 // nstrip
        for j in range(nstrip):
            lo, hi = j * sw, (j + 1) * sw
            nc.vector.tensor_scalar_mul(osb[:, lo:hi], po0[:, lo:hi], r[:])
            nc.scalar.dma_start(dram["out"][b, bass.ts(ti, P), lo:hi],
                                osb[:, lo:hi])
            nc.scalar.activation(osb[:, 512 + lo:512 + hi], po1[:, lo:hi],
                                 mybir.ActivationFunctionType.Copy, scale=r[:])
            nc.gpsimd.dma_start(dram["out"][b, bass.ts(ti, P), 512 + lo:512 + hi],
                                osb[:, 512 + lo:512 + hi])


def _build_nc():
    nc = bass.Bass()
    dram = {
        "qT": nc.declare_dram_parameter("qT", [BPC, E, T], F16, isOutput=False),
        "kT": nc.declare_dram_parameter("kT", [BPC, E, T], F16, isOutput=False),
        "vT": nc.declare_dram_parameter("vT", [BPC, E, T], F16, isOutput=False),
        "g": nc.declare_dram_parameter("g", [E, E], F16, isOutput=False),
        "wv": nc.declare_dram_parameter("wv", [E, H], F16, isOutput=False),
        "pad": nc.declare_dram_parameter("pad", [BPC, T, 1], F16, isOutput=False),
        "out": nc.declare_dram_parameter("out", [BPC, T, H], F32, isOutput=True),
    }
    with tile.TileContext(nc) as tc, ExitStack() as ctx:
        sb = ctx.enter_context(tc.tile_pool(name="sb", bufs=1))
        ps = ctx.enter_context(tc.tile_pool(name="ps", bufs=6, space="PSUM"))
        psd = ctx.enter_context(tc.tile_pool(name="psd", bufs=2, space="PSUM"))

        pools = {"sb": sb, "ps": ps, "psd": psd}
        pools["g"] = [sb.tile([P, E], F16, name=f"g{ec}") for ec in range(NC)]
        pools["wv"] = [sb.tile([P, H], F16, name=f"wv{ec}") for ec in range(NC)]

        # PE warm-up: dependency-free junk matmuls bridge the initial DMA
        # window (~14 us: DGE priming + first operand transfers) and trip
        # the HAM clock gate to 2.4 GHz before the first real matmul.  Four
        # rotating PSUM tiles, one long accumulation group per tile, so no
        # WAW semaphores serialize the stream.
        NWARM = 32
        warm = sb.tile([P, 512], F16, name="warm")
        nc.vector.memset(warm[:], 0.0)
        wps = [ps.tile([P, 512], F32, name="ps") for _ in range(4)]
        for i in range(NWARM):
            nc.tensor.matmul(wps[i % 4][:], lhsT=warm[:, 0:P], rhs=warm[:],
                             start=(i < 4), stop=(i >= NWARM - 4),
                             skip_group_check=True)

        for b in range(BPC):
            _emit_batch(nc, pools, b, dram)

    _split_multi_waits(nc)
    return nc


def _get_nc():
    global _nc_cache
    if _nc_cache is None:
        _nc_cache = _build_nc()
    return _nc_cache


def _make_in_maps(key, query, value, padding_mask, Wk, Wq, Wv):
    f16 = np.float16
    g = (np.asarray(Wq, np.float32).T @ np.asarray(Wk, np.float32)).astype(f16)
    wv = np.ascontiguousarray(np.asarray(Wv, np.float32).T).astype(f16)
    pad01 = (padding_mask.reshape(NB, T) == 0).astype(np.float32)  # [B,T]
    in_maps = []
    for c in range(NCORES):
        s = slice(BPC * c, BPC * (c + 1))
        qT = np.ascontiguousarray(query[s].transpose(0, 2, 1)).astype(f16)
        kT = np.ascontiguousarray(key[s].transpose(0, 2, 1)).astype(f16)
        vTf = value[s].transpose(0, 2, 1) * pad01[s][:, None, :]
        vT = np.ascontiguousarray(vTf).astype(f16)
        in_maps.append({
            "qT": qT, "kT": kT, "vT": vT,
            "g": g, "wv": wv,
            "pad": pad01[s].astype(f16).reshape(BPC, T, 1),
        })
    return in_maps


def run_on_cores(in_maps, trace=False, **kw):
    nc = _get_nc()
    return run_bass_kernel_spmd(nc, in_maps, list(range(NCORES)), trace=trace, **kw)


def kernel(key, query, value, padding_mask, Wk, Wq, Wv):
    key = np.asarray(key)
    query = np.asarray(query)
    value = np.asarray(value)
    padding_mask = np.asarray(padding_mask)
    in_maps = _make_in_maps(key, query, value, padding_mask,
                            np.asarray(Wk), np.asarray(Wq), np.asarray(Wv))
    res = run_on_cores(in_maps)
    out = np.empty((NB, T, H), np.float32)
    for c in range(NCORES):
        out[BPC * c: BPC * (c + 1)] = res.results[c]["out"]
    return out


# revision 15
# speedup vs baseline: 1.0784x; 1.0381x over previous
"""Causal (diagonal=1) single-head attention for trn2, 8-core SPMD.

Reference computation (fp32):
    k = key @ Wk.T; q = query @ Wq.T; v = value @ Wv.T       # [B,T,H]
    qk = (q @ k.T) / sqrt(E)                                  # [B,T,T]
    qk masked with tril(ones, k=1) and padding_mask           # -inf outside
    attn = softmax(qk, -1) @ v                                # [B,T,H]

Sharding: data-parallel over batch, 2 batches per core, no collectives.

Algebraic cut: q @ k.T = (query @ Wq.T)(key @ Wk.T).T
            = query @ (Wq.T @ Wk) @ key.T = query @ G @ key.T,
with G = Wq.T @ Wk precomputed once on the host.  The device then runs
ONE projection (A = query @ G) instead of two, and the scores matmul
consumes keyT straight from its DMA layout -- the whole k-projection
(1024^3 MACs/batch) disappears.

Device kernel (per core, per batch), all matmuls fp16 with fp32 PSUM
(fp16's 10 mantissa bits beat bf16's 8; same PE throughput):
    AT[e,t]  = sum_ec G[ec][:,e-chunk].T @ queryT[ec][:,t]    (proj)
    v[s,h]   = valueT[ec][:,s-chunk].T @ Wv.T[ec][:,h]
    sT[s,t]  = keyT-chunk.T @ AT  (only causally-live s-chunks)
    pT[s,t]  = exp(sT/32)  (ScalarE; max-subtraction skipped: |s/32| <~ 6)
    pT       = affine_select(pT, keep j<=i+1, else 0)         (GPSIMD)
    num[t,h] = pT-chunk.T @ v ; den[t,1] = pT-chunk.T @ pad01
    out[t,h] = num * reciprocal(den)                          (VectorE)

padding_mask is folded in exactly on the host: v rows and the denominator
column are scaled by pad01 = (padding_mask == 0), which equals softmax
with -inf at padded keys.
"""
from contextlib import ExitStack

import numpy as np

import concourse.bass as bass
import concourse.mybir as mybir
import concourse.tile as tile
from concourse.bass_utils import run_bass_kernel_spmd

F16 = mybir.dt.float16
F32 = mybir.dt.float32
P = 128
T = 1024           # sequence length
E = 1024           # embed dim
H = 1024           # head dim
NB = 16            # full batch
NCORES = 8
BPC = NB // NCORES  # batches per core
NC = T // P        # 128-chunks per dim (8)
SCALE = 1.0 / 32.0  # 1/sqrt(E)

_nc_cache = None


# --- walrus workaround: one sync-wait per instruction ---------------------
def _split_multi_waits(nc):
    """This walrus build rejects instructions with >1 sync wait (2 for
    EventSemaphore).  Move extra waits onto fresh same-engine NOPs placed
    immediately before the instruction; per-engine in-order execution
    preserves the gating, and semaphore updates stay on the original."""
    for fn in nc.m.functions:
        for bb in fn.blocks:
            il = bb.instructions
            idx = 0
            while idx < len(il):
                inst = il[idx]
                si = inst.sync_info
                waits = list(si.on_wait) if si and si.on_wait else []
                cap = 2 if isinstance(inst, mybir.InstEventSemaphore) else 1
                if len(waits) > cap:
                    extra, keep = waits[:-cap], waits[-cap:]
                    for j, w in enumerate(extra):
                        nop = mybir.InstNoOp(
                            name=f"I-wsplit-{inst.name}-{j}",
                            engine=inst.engine,
                            ins=[],
                            outs=[],
                            sync_info=mybir.SyncInfo(on_wait=[w], on_update=[]),
                        )
                        il.insert(idx, nop)
                        idx += 1
                    inst.sync_info = mybir.SyncInfo(
                        on_wait=keep, on_update=list(si.on_update or [])
                    )
                idx += 1


def _n_sc(ti):
    """Number of live 128-wide s-chunks for t-tile ti (cols j <= t+1)."""
    return min(ti + 2, NC)


def _emit_batch(nc, pools, b, dram):
    Exp = mybir.ActivationFunctionType.Exp
    g_t, w_v = pools["g"], pools["wv"]
    sb, ps, psd = pools["sb"], pools["ps"], pools["psd"]
    first = b == 0

    # -- input loads, spread across engine DMA queues so transfers run in
    #    parallel (each engine's dma_start lands on its own DGE queue and an
    #    engine-issued DMA occupies that engine for the transfer time):
    #      sync:   queryT halves, then keyT, then next batch's loads
    #      vector: G slices ht-major (batch 0), then Wv
    #      gpsimd: valueT st-major slices, padding col
    #    Ordered so the first A-proj group's operands land first. --
    # query halves go to separate tiles: a single split-loaded tile would
    # give the high-half DMA a tile-granular WAR hazard against the tg=0
    # matmuls and serialize the loads behind the compute
    qlo = [sb.tile([P, 512], F16, name=f"qlo{ec}") for ec in range(NC)]
    qhi = [sb.tile([P, 512], F16, name=f"qhi{ec}") for ec in range(NC)]
    for ec in range(NC):
        nc.sync.dma_start(qlo[ec][:], dram["qT"][b, bass.ts(ec, P), 0:512])
    if first:
        # whole [128,1024] chunks (2 KB DMA lines; 128-col slices would run
        # at ~1/5 the bandwidth), split 6/2 over gpsimd+sync so both queues
        # finish together just as the warm-up junk drains
        for ec in range(6):
            nc.gpsimd.dma_start(g_t[ec][:], dram["g"][bass.ts(ec, P), :])
        for ec in range(6, NC):
            nc.sync.dma_start(g_t[ec][:], dram["g"][bass.ts(ec, P), :])
    for ec in range(NC):
        nc.sync.dma_start(qhi[ec][:], dram["qT"][b, bass.ts(ec, P), 512:1024])
    if first:
        # Wv rides sync's ordered queue behind the query loads (an idle
        # engine's queue would hoist it into the critical head window)
        for ec in range(NC):
            nc.sync.dma_start(w_v[ec][:], dram["wv"][bass.ts(ec, P), :])

    # -- A = query @ G, transposed chunks AT[ht] --
    ATs = [sb.tile([P, T], F16, name=f"ATs{h}") for h in range(NC)]
    for tg, qx in ((0, qlo), (1, qhi)):
        for ht in range(NC):
            acc = ps.tile([P, 512], F32, name="ps")
            for ec in range(NC):
                nc.tensor.matmul(
                    acc[:],
                    lhsT=g_t[ec][:, bass.ts(ht, P)],
                    rhs=qx[ec][:],
                    start=(ec == 0),
                    stop=(ec == NC - 1),
                )
            nc.scalar.copy(ATs[ht][:, bass.ts(tg, 512)], acc[:])

    # -- v = value @ Wv.T --
    vin = [sb.tile([P, T], F16, name=f"vin{ec}") for ec in range(NC)]
    for ec in range(NC):
        nc.gpsimd.dma_start(vin[ec][:], dram["vT"][b, bass.ts(ec, P), :])
    kin = [sb.tile([P, T], F16, name=f"kin{ec}") for ec in range(NC)]
    for ec in range(NC):
        nc.sync.dma_start(kin[ec][:], dram["kT"][b, bass.ts(ec, P), :])
    padt = sb.tile([P, NC], F16, name="padt", bufs=2)
    nc.gpsimd.dma_start(
        padt[:], dram["pad"][b].rearrange("(c p) x -> p (c x)", p=P)
    )
    v_sb = [sb.tile([P, T], F16, name=f"vsb{s}") for s in range(NC)]
    for st in range(NC):
        for hh in range(2):
            acc = ps.tile([P, 512], F32, name="ps")
            for ec in range(NC):
                nc.tensor.matmul(
                    acc[:],
                    lhsT=vin[ec][:, bass.ts(st, P)],
                    rhs=w_v[ec][:, bass.ts(hh, 512)],
                    start=(ec == 0),
                    stop=(ec == NC - 1),
                )
            nc.vector.tensor_copy(v_sb[st][:, bass.ts(hh, 512)], acc[:])

    # -- scores^T + exp + causal zeroing --
    # Ragged t-groups aligned to where the live s-chunk count jumps
    # (t = 128k - 1 because of the +1 diagonal): 2/4/6/8 live chunks per
    # group = 20 column-blocks vs 23 for aligned 256-wide groups.
    BOUNDS = (0, 255, 511, 767, 1024)
    pT = [sb.tile([P, T], F16, name=f"pT{s}") for s in range(NC)]
    # Blocks that are causally dead but still read by the attn matmuls
    # (created by the ragged grouping) must be zeroed explicitly.
    for sc, pt0, pt1 in ((2, 128, 255), (4, 384, 511), (6, 640, 767)):
        nc.vector.memset(pT[sc][:, pt0:pt1], 0.0)
    for g in range(4):
        t0, t1 = BOUNDS[g], BOUNDS[g + 1]
        w = t1 - t0
        for sc in range(2 * g + 2):
            acc = ps.tile([P, 512], F32, name="ps")
            for ec in range(NC):
                nc.tensor.matmul(
                    acc[:, :w],
                    lhsT=kin[ec][:, bass.ts(sc, P)],
                    rhs=ATs[ec][:, t0:t1],
                    start=(ec == 0),
                    stop=(ec == NC - 1),
                )
            dst = pT[sc][:, t0:t1]
            nc.scalar.activation(dst, acc[:, :w], Exp, scale=SCALE)
            off = 128 * sc - t0
            if off >= -125:
                # keep where t_local - s_local - off + 1 >= 0 (j <= i+1)
                nc.gpsimd.affine_select(
                    out=dst,
                    in_=dst,
                    compare_op=mybir.AluOpType.is_ge,
                    fill=0.0,
                    base=1 - off,
                    pattern=[[1, w]],
                    channel_multiplier=-1,
                )

    # -- attn = (pT.T @ [v, pad01]) with post-normalization --
    for ti in range(NC):
        nsc = _n_sc(ti)
        po0 = ps.tile([P, 512], F32, name="ps")
        po1 = ps.tile([P, 512], F32, name="ps")
        pd = psd.tile([P, 1], F32, name="psd")
        for sc in range(nsc):
            lhsT = pT[sc][:, bass.ts(ti, P)]
            st, sp = (sc == 0), (sc == nsc - 1)
            nc.tensor.matmul(po0[:], lhsT=lhsT, rhs=v_sb[sc][:, 0:512],
                             start=st, stop=sp)
            nc.tensor.matmul(po1[:], lhsT=lhsT, rhs=v_sb[sc][:, 512:1024],
                             start=st, stop=sp)
            nc.tensor.matmul(pd[:], lhsT=lhsT, rhs=padt[:, sc:sc + 1],
                             start=st, stop=sp)
        r = sb.tile([P, 1], F32, name="recip", bufs=3)
        nc.vector.reciprocal(r[:], pd[:])
        osb = sb.tile([P, T], F32, name="osb", bufs=3)
        # the two halves scale concurrently on VectorE and ScalarE; the
        # output DMA is spread over three queues (gpsimd takes the high
        # half, sync/scalar alternate the low half) so no single queue
        # exceeds what it can sustain during the attn phase
        # final batch's low halves ride sync (idle by then) so the very
        # last tile's two DMAs run concurrently on separate queues
        lo_eng = nc.sync if b == BPC - 1 else nc.scalar
        nc.vector.tensor_scalar_mul(osb[:, 0:512], po0[:], r[:])
        lo_eng.dma_start(dram["out"][b, bass.ts(ti, P), 0:512], osb[:, 0:512])
        nc.scalar.activation(osb[:, 512:1024], po1[:],
                             mybir.ActivationFunctionType.Copy, scale=r[:])
        nc.gpsimd.dma_start(dram["out"][b, bass.ts(ti, P), 512:1024],
                            osb[:, 512:1024])


def _build_nc():
    nc = bass.Bass()
    dram = {
        "qT": nc.declare_dram_parameter("qT", [BPC, E, T], F16, isOutput=False),
        "kT": nc.declare_dram_parameter("kT", [BPC, E, T], F16, isOutput=False),
        "vT": nc.declare_dram_parameter("vT", [BPC, E, T], F16, isOutput=False),
        "g": nc.declare_dram_parameter("g", [E, E], F16, isOutput=False),
        "wv": nc.declare_dram_parameter("wv", [E, H], F16, isOutput=False),
        "pad": nc.declare_dram_parameter("pad", [BPC, T, 1], F16, isOutput=False),
        "out": nc.declare_dram_parameter("out", [BPC, T, H], F32, isOutput=True),
    }
    with tile.TileContext(nc) as tc, ExitStack() as ctx:
        sb = ctx.enter_context(tc.tile_pool(name="sb", bufs=1))
        ps = ctx.enter_context(tc.tile_pool(name="ps", bufs=6, space="PSUM"))
        psd = ctx.enter_context(tc.tile_pool(name="psd", bufs=2, space="PSUM"))

        pools = {"sb": sb, "ps": ps, "psd": psd}
        pools["g"] = [sb.tile([P, E], F16, name=f"g{ec}") for ec in range(NC)]
        pools["wv"] = [sb.tile([P, H], F16, name=f"wv{ec}") for ec in range(NC)]

        # PE warm-up: dependency-free junk matmuls bridge the initial DMA
        # window (~14 us: DGE priming + first operand transfers) and trip
        # the HAM clock gate to 2.4 GHz before the first real matmul.  Four
        # rotating PSUM tiles, one long accumulation group per tile, so no
        # WAW semaphores serialize the stream.
        NWARM = 32
        warm = sb.tile([P, 512], F16, name="warm")
        nc.vector.memset(warm[:], 0.0)
        wps = [ps.tile([P, 512], F32, name="ps") for _ in range(4)]
        for i in range(NWARM):
            nc.tensor.matmul(wps[i % 4][:], lhsT=warm[:, 0:P], rhs=warm[:],
                             start=(i < 4), stop=(i >= NWARM - 4),
                             skip_group_check=True)

        for b in range(BPC):
            _emit_batch(nc, pools, b, dram)

    _split_multi_waits(nc)
    return nc


def _get_nc():
    global _nc_cache
    if _nc_cache is None:
        _nc_cache = _build_nc()
    return _nc_cache


def _make_in_maps(key, query, value, padding_mask, Wk, Wq, Wv):
    f16 = np.float16
    g = (np.asarray(Wq, np.float32).T @ np.asarray(Wk, np.float32)).astype(f16)
    wv = np.ascontiguousarray(np.asarray(Wv, np.float32).T).astype(f16)
    pad01 = (padding_mask.reshape(NB, T) == 0).astype(np.float32)  # [B,T]
    in_maps = []
    for c in range(NCORES):
        s = slice(BPC * c, BPC * (c + 1))
        qT = np.ascontiguousarray(query[s].transpose(0, 2, 1)).astype(f16)
        kT = np.ascontiguousarray(key[s].transpose(0, 2, 1)).astype(f16)
        vTf = value[s].transpose(0, 2, 1) * pad01[s][:, None, :]
        vT = np.ascontiguousarray(vTf).astype(f16)
        in_maps.append({
            "qT": qT, "kT": kT, "vT": vT,
            "g": g, "wv": wv,
            "pad": pad01[s].astype(f16).reshape(BPC, T, 1),
        })
    return in_maps


def run_on_cores(in_maps, trace=False, **kw):
    nc = _get_nc()
    return run_bass_kernel_spmd(nc, in_maps, list(range(NCORES)), trace=trace, **kw)


def kernel(key, query, value, padding_mask, Wk, Wq, Wv):
    key = np.asarray(key)
    query = np.asarray(query)
    value = np.asarray(value)
    padding_mask = np.asarray(padding_mask)
    in_maps = _make_in_maps(key, query, value, padding_mask,
                            np.asarray(Wk), np.asarray(Wq), np.asarray(Wv))
    res = run_on_cores(in_maps)
    out = np.empty((NB, T, H), np.float32)
    for c in range(NCORES):
        out[BPC * c: BPC * (c + 1)] = res.results[c]["out"]
    return out


# revision 18
# speedup vs baseline: 1.0797x; 1.0012x over previous
"""Causal (diagonal=1) single-head attention for trn2, 8-core SPMD.

Reference computation (fp32):
    k = key @ Wk.T; q = query @ Wq.T; v = value @ Wv.T       # [B,T,H]
    qk = (q @ k.T) / sqrt(E)                                  # [B,T,T]
    qk masked with tril(ones, k=1) and padding_mask           # -inf outside
    attn = softmax(qk, -1) @ v                                # [B,T,H]

Sharding: data-parallel over batch, 2 batches per core, no collectives.

Algebraic cut: q @ k.T = (query @ Wq.T)(key @ Wk.T).T
            = query @ (Wq.T @ Wk) @ key.T = query @ G @ key.T,
with G = Wq.T @ Wk precomputed once on the host.  The device then runs
ONE projection (A = query @ G) instead of two, and the scores matmul
consumes keyT straight from its DMA layout -- the whole k-projection
(1024^3 MACs/batch) disappears.

Device kernel (per core, per batch), all matmuls fp16 with fp32 PSUM
(fp16's 10 mantissa bits beat bf16's 8; same PE throughput):
    AT[e,t]  = sum_ec G[ec][:,e-chunk].T @ queryT[ec][:,t]    (proj)
    v[s,h]   = valueT[ec][:,s-chunk].T @ Wv.T[ec][:,h]
    sT[s,t]  = keyT-chunk.T @ AT  (only causally-live s-chunks)
    pT[s,t]  = exp(sT/32)  (ScalarE; max-subtraction skipped: |s/32| <~ 6)
    pT       = affine_select(pT, keep j<=i+1, else 0)         (GPSIMD)
    num[t,h] = pT-chunk.T @ v ; den[t,1] = pT-chunk.T @ pad01
    out[t,h] = num * reciprocal(den)                          (VectorE)

padding_mask is folded in exactly on the host: v rows and the denominator
column are scaled by pad01 = (padding_mask == 0), which equals softmax
with -inf at padded keys.
"""
from contextlib import ExitStack

import numpy as np

import concourse.bass as bass
import concourse.mybir as mybir
import concourse.tile as tile
from concourse.bass_utils import run_bass_kernel_spmd

F16 = mybir.dt.float16
F32 = mybir.dt.float32
P = 128
T = 1024           # sequence length
E = 1024           # embed dim
H = 1024           # head dim
NB = 16            # full batch
NCORES = 8
BPC = NB // NCORES  # batches per core
NC = T // P        # 128-chunks per dim (8)
SCALE = 1.0 / 32.0  # 1/sqrt(E)

_nc_cache = None


# --- walrus workaround: one sync-wait per instruction ---------------------
def _split_multi_waits(nc):
    """This walrus build rejects instructions with >1 sync wait (2 for
    EventSemaphore).  Move extra waits onto fresh same-engine NOPs placed
    immediately before the instruction; per-engine in-order execution
    preserves the gating, and semaphore updates stay on the original."""
    for fn in nc.m.functions:
        for bb in fn.blocks:
            il = bb.instructions
            idx = 0
            while idx < len(il):
                inst = il[idx]
                si = inst.sync_info
                waits = list(si.on_wait) if si and si.on_wait else []
                cap = 2 if isinstance(inst, mybir.InstEventSemaphore) else 1
                if len(waits) > cap:
                    extra, keep = waits[:-cap], waits[-cap:]
                    for j, w in enumerate(extra):
                        nop = mybir.InstNoOp(
                            name=f"I-wsplit-{inst.name}-{j}",
                            engine=inst.engine,
                            ins=[],
                            outs=[],
                            sync_info=mybir.SyncInfo(on_wait=[w], on_update=[]),
                        )
                        il.insert(idx, nop)
                        idx += 1
                    inst.sync_info = mybir.SyncInfo(
                        on_wait=keep, on_update=list(si.on_update or [])
                    )
                idx += 1


def _n_sc(ti):
    """Number of live 128-wide s-chunks for t-tile ti (cols j <= t+1)."""
    return min(ti + 2, NC)


def _emit_batch(nc, pools, b, dram):
    Exp = mybir.ActivationFunctionType.Exp
    g_t, w_v = pools["g"], pools["wv"]
    sb, ps, psd = pools["sb"], pools["ps"], pools["psd"]
    first = b == 0

    # -- input loads, spread across engine DMA queues so transfers run in
    #    parallel (each engine's dma_start lands on its own DGE queue and an
    #    engine-issued DMA occupies that engine for the transfer time):
    #      sync:   queryT halves, then keyT, then next batch's loads
    #      vector: G slices ht-major (batch 0), then Wv
    #      gpsimd: valueT st-major slices, padding col
    #    Ordered so the first A-proj group's operands land first. --
    # query halves go to separate tiles: a single split-loaded tile would
    # give the high-half DMA a tile-granular WAR hazard against the tg=0
    # matmuls and serialize the loads behind the compute
    qlo = [sb.tile([P, 512], F16, name=f"qlo{ec}") for ec in range(NC)]
    qhi = [sb.tile([P, 512], F16, name=f"qhi{ec}") for ec in range(NC)]
    if first:
        # whole [128,1024] chunks (2 KB DMA lines; 128-col slices would run
        # at ~1/5 the bandwidth), split 6/2 over gpsimd+sync so both queues
        # finish together just as the warm-up junk drains.  G before the
        # query halves: every first-group matmul needs G.
        for ec in range(6):
            nc.gpsimd.dma_start(g_t[ec][:], dram["g"][bass.ts(ec, P), :])
        for ec in range(6, NC):
            nc.sync.dma_start(g_t[ec][:], dram["g"][bass.ts(ec, P), :])
    for ec in range(NC):
        nc.sync.dma_start(qlo[ec][:], dram["qT"][b, bass.ts(ec, P), 0:512])
    for ec in range(NC):
        nc.sync.dma_start(qhi[ec][:], dram["qT"][b, bass.ts(ec, P), 512:1024])
    if first:
        # Wv rides sync's ordered queue behind the query loads (an idle
        # engine's queue would hoist it into the critical head window)
        for ec in range(NC):
            nc.sync.dma_start(w_v[ec][:], dram["wv"][bass.ts(ec, P), :])

    # -- A = query @ G, transposed chunks AT[ht] --
    ATs = [sb.tile([P, T], F16, name=f"ATs{h}") for h in range(NC)]
    for tg, qx in ((0, qlo), (1, qhi)):
        for ht in range(NC):
            acc = ps.tile([P, 512], F32, name="ps")
            for ec in range(NC):
                nc.tensor.matmul(
                    acc[:],
                    lhsT=g_t[ec][:, bass.ts(ht, P)],
                    rhs=qx[ec][:],
                    start=(ec == 0),
                    stop=(ec == NC - 1),
                )
            nc.scalar.copy(ATs[ht][:, bass.ts(tg, 512)], acc[:])

    # -- v = value @ Wv.T --
    vin = [sb.tile([P, T], F16, name=f"vin{ec}") for ec in range(NC)]
    for ec in range(NC):
        nc.gpsimd.dma_start(vin[ec][:], dram["vT"][b, bass.ts(ec, P), :])
    kin = [sb.tile([P, T], F16, name=f"kin{ec}") for ec in range(NC)]
    for ec in range(NC):
        nc.sync.dma_start(kin[ec][:], dram["kT"][b, bass.ts(ec, P), :])
    padt = sb.tile([P, NC], F16, name="padt", bufs=2)
    nc.gpsimd.dma_start(
        padt[:], dram["pad"][b].rearrange("(c p) x -> p (c x)", p=P)
    )
    v_sb = [sb.tile([P, T], F16, name=f"vsb{s}") for s in range(NC)]
    for st in range(NC):
        for hh in range(2):
            acc = ps.tile([P, 512], F32, name="ps")
            for ec in range(NC):
                nc.tensor.matmul(
                    acc[:],
                    lhsT=vin[ec][:, bass.ts(st, P)],
                    rhs=w_v[ec][:, bass.ts(hh, 512)],
                    start=(ec == 0),
                    stop=(ec == NC - 1),
                )
            nc.vector.tensor_copy(v_sb[st][:, bass.ts(hh, 512)], acc[:])

    # -- scores^T + exp + causal zeroing --
    # Ragged t-groups aligned to where the live s-chunk count jumps
    # (t = 128k - 1 because of the +1 diagonal): 2/4/6/8 live chunks per
    # group = 20 column-blocks vs 23 for aligned 256-wide groups.
    BOUNDS = (0, 255, 511, 767, 1024)
    pT = [sb.tile([P, T], F16, name=f"pT{s}") for s in range(NC)]
    # Blocks that are causally dead but still read by the attn matmuls
    # (created by the ragged grouping) must be zeroed explicitly.
    for sc, pt0, pt1 in ((2, 128, 255), (4, 384, 511), (6, 640, 767)):
        nc.vector.memset(pT[sc][:, pt0:pt1], 0.0)
    for g in range(4):
        t0, t1 = BOUNDS[g], BOUNDS[g + 1]
        w = t1 - t0
        for sc in range(2 * g + 2):
            acc = ps.tile([P, 512], F32, name="ps")
            for ec in range(NC):
                nc.tensor.matmul(
                    acc[:, :w],
                    lhsT=kin[ec][:, bass.ts(sc, P)],
                    rhs=ATs[ec][:, t0:t1],
                    start=(ec == 0),
                    stop=(ec == NC - 1),
                )
            dst = pT[sc][:, t0:t1]
            nc.scalar.activation(dst, acc[:, :w], Exp, scale=SCALE)
            off = 128 * sc - t0
            if off >= -125:
                # keep where t_local - s_local - off + 1 >= 0 (j <= i+1)
                nc.gpsimd.affine_select(
                    out=dst,
                    in_=dst,
                    compare_op=mybir.AluOpType.is_ge,
                    fill=0.0,
                    base=1 - off,
                    pattern=[[1, w]],
                    channel_multiplier=-1,
                )

    # -- attn = (pT.T @ [v, pad01]) with post-normalization --
    for ti in range(NC):
        nsc = _n_sc(ti)
        po0 = ps.tile([P, 512], F32, name="ps")
        po1 = ps.tile([P, 512], F32, name="ps")
        pd = psd.tile([P, 1], F32, name="psd")
        for sc in range(nsc):
            lhsT = pT[sc][:, bass.ts(ti, P)]
            st, sp = (sc == 0), (sc == nsc - 1)
            nc.tensor.matmul(po0[:], lhsT=lhsT, rhs=v_sb[sc][:, 0:512],
                             start=st, stop=sp)
            nc.tensor.matmul(po1[:], lhsT=lhsT, rhs=v_sb[sc][:, 512:1024],
                             start=st, stop=sp)
            nc.tensor.matmul(pd[:], lhsT=lhsT, rhs=padt[:, sc:sc + 1],
                             start=st, stop=sp)
        r = sb.tile([P, 1], F32, name="recip", bufs=3)
        nc.vector.reciprocal(r[:], pd[:])
        osb = sb.tile([P, T], F16, name="osb", bufs=3)
        # the two halves scale concurrently on VectorE and ScalarE; the
        # output DMA is spread over three queues (gpsimd takes the high
        # half, sync/scalar alternate the low half) so no single queue
        # exceeds what it can sustain during the attn phase
        # final batch's low halves ride sync (idle by then) so the very
        # last tile's two DMAs run concurrently on separate queues
        lo_eng = nc.sync if b == BPC - 1 else nc.scalar
        nc.vector.tensor_scalar_mul(osb[:, 0:512], po0[:], r[:])
        lo_eng.dma_start(dram["out"][b, bass.ts(ti, P), 0:512], osb[:, 0:512])
        nc.scalar.activation(osb[:, 512:1024], po1[:],
                             mybir.ActivationFunctionType.Copy, scale=r[:])
        nc.gpsimd.dma_start(dram["out"][b, bass.ts(ti, P), 512:1024],
                            osb[:, 512:1024])


def _build_nc():
    nc = bass.Bass()
    dram = {
        "qT": nc.declare_dram_parameter("qT", [BPC, E, T], F16, isOutput=False),
        "kT": nc.declare_dram_parameter("kT", [BPC, E, T], F16, isOutput=False),
        "vT": nc.declare_dram_parameter("vT", [BPC, E, T], F16, isOutput=False),
        "g": nc.declare_dram_parameter("g", [E, E], F16, isOutput=False),
        "wv": nc.declare_dram_parameter("wv", [E, H], F16, isOutput=False),
        "pad": nc.declare_dram_parameter("pad", [BPC, T, 1], F16, isOutput=False),
        "out": nc.declare_dram_parameter("out", [BPC, T, H], F16, isOutput=True),
    }
    with tile.TileContext(nc) as tc, ExitStack() as ctx:
        sb = ctx.enter_context(tc.tile_pool(name="sb", bufs=1))
        ps = ctx.enter_context(tc.tile_pool(name="ps", bufs=6, space="PSUM"))
        psd = ctx.enter_context(tc.tile_pool(name="psd", bufs=2, space="PSUM"))

        pools = {"sb": sb, "ps": ps, "psd": psd}
        pools["g"] = [sb.tile([P, E], F16, name=f"g{ec}") for ec in range(NC)]
        pools["wv"] = [sb.tile([P, H], F16, name=f"wv{ec}") for ec in range(NC)]

        # PE warm-up: dependency-free junk matmuls bridge the initial DMA
        # window (~14 us: DGE priming + first operand transfers) and trip
        # the HAM clock gate to 2.4 GHz before the first real matmul.  Four
        # rotating PSUM tiles, one long accumulation group per tile, so no
        # WAW semaphores serialize the stream.
        NWARM = 28
        warm = sb.tile([P, 512], F16, name="warm")
        nc.vector.memset(warm[:], 0.0)
        wps = [ps.tile([P, 512], F32, name="ps") for _ in range(4)]
        for i in range(NWARM):
            nc.tensor.matmul(wps[i % 4][:], lhsT=warm[:, 0:P], rhs=warm[:],
                             start=(i < 4), stop=(i >= NWARM - 4),
                             skip_group_check=True)

        for b in range(BPC):
            _emit_batch(nc, pools, b, dram)

    _split_multi_waits(nc)
    return nc


def _get_nc():
    global _nc_cache
    if _nc_cache is None:
        _nc_cache = _build_nc()
    return _nc_cache


def _make_in_maps(key, query, value, padding_mask, Wk, Wq, Wv):
    f16 = np.float16
    g = (np.asarray(Wq, np.float32).T @ np.asarray(Wk, np.float32)).astype(f16)
    wv = np.ascontiguousarray(np.asarray(Wv, np.float32).T).astype(f16)
    pad01 = (padding_mask.reshape(NB, T) == 0).astype(np.float32)  # [B,T]
    in_maps = []
    for c in range(NCORES):
        s = slice(BPC * c, BPC * (c + 1))
        qT = np.ascontiguousarray(query[s].transpose(0, 2, 1)).astype(f16)
        kT = np.ascontiguousarray(key[s].transpose(0, 2, 1)).astype(f16)
        vTf = value[s].transpose(0, 2, 1) * pad01[s][:, None, :]
        vT = np.ascontiguousarray(vTf).astype(f16)
        in_maps.append({
            "qT": qT, "kT": kT, "vT": vT,
            "g": g, "wv": wv,
            "pad": pad01[s].astype(f16).reshape(BPC, T, 1),
        })
    return in_maps


def run_on_cores(in_maps, trace=False, **kw):
    nc = _get_nc()
    return run_bass_kernel_spmd(nc, in_maps, list(range(NCORES)), trace=trace, **kw)


def kernel(key, query, value, padding_mask, Wk, Wq, Wv):
    key = np.asarray(key)
    query = np.asarray(query)
    value = np.asarray(value)
    padding_mask = np.asarray(padding_mask)
    in_maps = _make_in_maps(key, query, value, padding_mask,
                            np.asarray(Wk), np.asarray(Wq), np.asarray(Wv))
    res = run_on_cores(in_maps)
    out = np.empty((NB, T, H), np.float32)
    for c in range(NCORES):
        out[BPC * c: BPC * (c + 1)] = res.results[c]["out"].astype(np.float32)
    return out
